# revision 19
# baseline (speedup 1.0000x reference)
"""Trainium2 Bass kernel for BotNet-style sparse attention (4 heads, 64x64 map,
dh=128, decomposed 2D relative position bias).

Sharding: 8 cores = 4 heads x 2 query-halves. Each core computes its head's
q/k/v from the full fmap, builds the rel-pos bias row tensors on chip, and runs
flash-style attention in "transposed sim" orientation (keys on partitions,
queries on free dim) so no attention-matrix transposes are needed:

  simT[k, q] = K^T.T @ Q^T  (+ bias via indicator-matmul accumulation)
  expT = exp(SCALE * simT - 4)           (ACT, PSUM->SBUF fp16)
  outT[d, q] = sum_k V[k, d] * expT[k,q] (PSUM accumulation over key chunks)
  rowsum via DVE accumulate + ones-matmul partition reduce
  out = outT * (1/rowsum) broadcast      (K=1 outer-product matmul broadcast)

The rel-pos bias decomposes per query q=(hq,wq), key k=(hk,wk) as
  bias = Rh[q, hk-hq+63] + Rw[q, wk-wq+63]
computed as 64-wide slices of rel^T against query groups (by image row for the
height term, by wq residue class for the width term), then folded into sim via
one extra accumulating matmul against a 0/1 indicator matrix.

Per-core inputs are key-permuted (own query half first) so the SPMD graph is
identical across cores; all per-core differences live in the input data.
"""

import numpy as np
import ml_dtypes

C, H, W = 512, 64, 64
HEADS, DH = 4, 128
L = H * W           # 4096
NQ = L // 2         # 2048 queries per core
QB = 1024           # query block
SCALE = DH ** -0.5
NCORES = 8

_GRAPH = None


def _build_graph():
    from concourse import bacc
    import concourse.mybir as mybir
    import concourse.tile as tile

    f32 = mybir.dt.float32
    bf16 = mybir.dt.bfloat16
    fp16 = mybir.dt.float16
    EXPF = mybir.ActivationFunctionType.Exp

    nc = bacc.Bacc(None)

    fmap_p = nc.declare_dram_parameter("fmapc", [C, L], bf16, isOutput=False)
    wt_p = nc.declare_dram_parameter("wt", [C, 384], bf16, isOutput=False)
    relh_p = nc.declare_dram_parameter("relh", [128, 96], bf16, isOutput=False)
    relw_p = nc.declare_dram_parameter("relw", [128, 127], bf16, isOutput=False)
    ind_p = nc.declare_dram_parameter("ind", [128, L], bf16, isOutput=False)
    onesh_p = nc.declare_dram_parameter("onesh", [128, 128], fp16, isOutput=False)
    bias4_p = nc.declare_dram_parameter("bias4", [128, 1], f32, isOutput=False)
    out_p = nc.declare_dram_parameter("out", [128, NQ], f32, isOutput=True)

    with tile.TileContext(nc) as tc:
        with tc.tile_pool(name="const", bufs=1) as cpool, \
             tc.tile_pool(name="big", bufs=1) as big, \
             tc.tile_pool(name="work", bufs=2) as work:

            # ---- constants to SBUF (small; after the gate DMAs) ----
            relh_sb = cpool.tile([128, 96], bf16, name="relh_sb")
            relw_sb = cpool.tile([128, 127], bf16, name="relw_sb")
            ind_sb = cpool.tile([128, L], bf16, name="ind_sb")
            onesh_sb = cpool.tile([128, 128], fp16, name="onesh_sb")
            bias4_sb = cpool.tile([128, 1], f32, name="bias4_sb")

            # ---- weights first (small, unblock qkv matmuls), then fmap
            # t-major so each 1024-column stripe completes across all four
            # c-tiles early; spread across engine DMA queues for bandwidth ----
            F4 = [big.tile([128, L], bf16, name=f"F{c}") for c in range(4)]
            W4 = []
            for c in range(4):
                w = big.tile([128, 384], bf16, name=f"W{c}")
                nc.scalar.dma_start(out=w, in_=wt_p[c * 128:(c + 1) * 128, :])
                W4.append(w)
            # bulk loads ride the two HWDGE queues (sync/scalar); gpsimd
            # SWDGE is too slow for bulk and only carries the tiny constants
            dma_engs = [nc.sync, nc.scalar, nc.scalar, nc.sync]
            for h in range(2):
                for c in range(4):
                    dma_engs[c].dma_start(
                        out=F4[c][:, h * 512:(h + 1) * 512],
                        in_=fmap_p[c * 128:(c + 1) * 128, h * 512:(h + 1) * 512])
            for c in range(4):
                dma_engs[c].dma_start(
                    out=F4[c][:, 1024:2048],
                    in_=fmap_p[c * 128:(c + 1) * 128, 1024:2048])
            for c in range(4):
                nc.sync.dma_start(out=ind_sb[:, 0:1024], in_=ind_p[:, 0:1024]) \
                    if c == 0 else None
                dma_engs[c].dma_start(
                    out=F4[c][:, 2048:4096],
                    in_=fmap_p[c * 128:(c + 1) * 128, 2048:4096])
            for k in range(1, 4):
                nc.sync.dma_start(out=ind_sb[:, k * 1024:(k + 1) * 1024],
                                  in_=ind_p[:, k * 1024:(k + 1) * 1024])
            nc.gpsimd.dma_start(out=relh_sb, in_=relh_p[:, :])
            nc.gpsimd.dma_start(out=relw_sb, in_=relw_p[:, :])
            nc.gpsimd.dma_start(out=onesh_sb, in_=onesh_p[:, :])
            nc.gpsimd.dma_start(out=bias4_sb, in_=bias4_p[:, :])

            QT = big.tile([128, NQ], bf16, name="QT")
            KT = big.tile([128, L], bf16, name="KT")
            VTt = big.tile([128, L], bf16, name="VTt")
            Vn = big.tile([128, L], bf16, name="Vn")
            BT = big.tile([128, NQ], bf16, name="BT")

            # ---- phase A+B: qkv projection pipelined with fmap stripe DMAs;
            # bias matmuls emitted mid-stream as PE gap-filler ----
            with tc.tile_pool(name="psA", bufs=2, space="PSUM") as psA:
                def qkv_group(dst, col0, t, eng):
                    ps = psA.tile([128, 1024], f32, name="qkv_ps", tag="qkv", bufs=2)
                    for c in range(4):
                        for h in range(2):
                            nc.tensor.matmul(
                                ps[:, h * 512:(h + 1) * 512],
                                W4[c][:, col0:col0 + 128],
                                F4[c][:, t * 1024 + h * 512: t * 1024 + (h + 1) * 512],
                                start=(c == 0), stop=(c == 3))
                    if eng == "act":
                        nc.scalar.copy(dst[:, t * 1024:(t + 1) * 1024], ps)
                    else:
                        nc.vector.tensor_copy(dst[:, t * 1024:(t + 1) * 1024], ps)

                def bias_half(h1):
                    q0 = h1 * 1024
                    bh_ps = psA.tile([64, 1024], f32, name="bh_ps", tag="bias", bufs=2)
                    for r in range(16):
                        rr = h1 * 16 + r
                        nc.tensor.matmul(
                            bh_ps[:, r * 64:(r + 1) * 64],
                            relh_sb[:, 31 - rr:95 - rr],
                            QT[:, q0 + r * 64:q0 + (r + 1) * 64],
                            start=True, stop=True)
                    nc.vector.tensor_copy(BT[0:64, q0:q0 + 1024], bh_ps)
                    bw_ps = psA.tile([64, 1024], f32, name="bw_ps", tag="bias", bufs=2)
                    for w in range(64):
                        nc.tensor.matmul(
                            bw_ps[:, w * 16:(w + 1) * 16],
                            relw_sb[:, 63 - w:127 - w],
                            QT.rearrange("d (i w) -> d w i", w=64)[:, w, h1 * 16:(h1 + 1) * 16],
                            start=True, stop=True)
                    nc.vector.tensor_copy(
                        BT[64:128, q0:q0 + 1024].rearrange("p (i w) -> p i w", i=16, w=64),
                        bw_ps.rearrange("p (w i) -> p i w", w=64, i=16))

                for t in range(4):
                    if t < 2:
                        qkv_group(QT, 0, t, "dve")
                    qkv_group(KT, 128, t, "act")
                    qkv_group(VTt, 256, t, "dve")
                    for s in range(t * 8, t * 8 + 8):
                        nc.sync.dma_start_transpose(
                            Vn[:, s * 128:(s + 1) * 128],
                            VTt[:, s * 128:(s + 1) * 128])
                    if t == 1:
                        bias_half(0)
                        bias_half(1)

            # ---- phase C: attention main loop ----
            with tc.tile_pool(name="psC", bufs=1, space="PSUM") as psC:
                for qb in range(2):
                    q0 = qb * QB
                    acc = work.tile([128, QB], fp16, name="acc", tag="acc", bufs=2)
                    outT = psC.tile([128, QB], f32, name="outT", tag="out", bufs=1)
                    for kc in range(32):
                        sim = psC.tile([128, QB], f32, name="sim", tag="sim", bufs=3)
                        for h in range(2):
                            sl = slice(q0 + h * 512, q0 + (h + 1) * 512)
                            po = sim[:, h * 512:(h + 1) * 512]
                            nc.tensor.matmul(
                                po, KT[:, kc * 128:(kc + 1) * 128], QT[:, sl],
                                start=True, stop=False)
                            nc.tensor.matmul(
                                po, ind_sb[:, kc * 128:(kc + 1) * 128], BT[:, sl],
                                start=False, stop=True)
                        expT = work.tile([128, QB], fp16, name="expT", tag="exp", bufs=4)
                        nc.scalar.activation(expT, sim, EXPF, bias=bias4_sb[:, 0:1], scale=SCALE)
                        if kc == 0:
                            nc.vector.tensor_copy(acc, expT)
                        else:
                            nc.vector.tensor_add(acc, acc, expT)
                        for h in range(2):
                            nc.tensor.matmul(
                                outT[:, h * 512:(h + 1) * 512],
                                Vn[:, kc * 128:(kc + 1) * 128],
                                expT[:, h * 512:(h + 1) * 512],
                                start=(kc == 0), stop=(kc == 31))

                    # rowsum: partition-reduce acc via ones-matmul
                    rs_ps = psC.tile([1, QB], f32, name="rs_ps", tag="sim", bufs=3)
                    for h in range(2):
                        nc.tensor.matmul(
                            rs_ps[:, h * 512:(h + 1) * 512],
                            onesh_sb[:, 0:1], acc[:, h * 512:(h + 1) * 512],
                            start=True, stop=True)
                    rs_row = work.tile([1, QB], fp16, name="rs_row", tag="rsrow")
                    nc.vector.tensor_copy(rs_row, rs_ps)
                    # broadcast rowsum across partitions (K=1 outer product),
                    # then wide approximate reciprocal and scale
                    bc_ps = psC.tile([128, QB], f32, name="bc_ps", tag="sim", bufs=3)
                    for hh in range(2):
                        nc.tensor.matmul(
                            bc_ps[:, hh * 512:(hh + 1) * 512],
                            onesh_sb[0:1, :], rs_row[0:1, hh * 512:(hh + 1) * 512],
                            start=True, stop=True)
                    rec_sb = work.tile([128, QB], f32, name="rec_sb", tag="bc")
                    nc.vector.reciprocal_approx_fast(out=rec_sb, in_=bc_ps)
                    out_sb = work.tile([128, QB], f32, name="out_sb", tag="osb")
                    nc.vector.tensor_mul(out_sb, outT, rec_sb)
                    nc.sync.dma_start(out=out_p[:, q0:q0 + QB], in_=out_sb)

    nc.finalize()
    return nc


def _prep_core_inputs(fmap, w_qkv, rel_height, rel_width, core):
    bf = ml_dtypes.bfloat16
    h, half = core // 2, core % 2
    q0 = half * NQ
    perm = (np.arange(L) + q0) % L
    fmap_flat = fmap.reshape(C, L)
    fmap_core = np.ascontiguousarray(fmap_flat[:, perm]).astype(bf)
    rows = np.r_[h * 128:(h + 1) * 128,
                 512 + h * 128:512 + (h + 1) * 128,
                 1024 + h * 128:1024 + (h + 1) * 128]
    wt = np.ascontiguousarray(w_qkv[rows].T).astype(bf)
    relhT = rel_height.T  # (128, 127)
    a = 32 * (1 - half)
    relh_slab = np.zeros((128, 96), np.float32)
    relh_slab[:, :95] = relhT[:, a:a + 95]
    relw = np.ascontiguousarray(rel_width.T).astype(bf)
    j = np.arange(L)
    ind = np.zeros((128, L), np.float32)
    ind[(j // 64 + 32 * half) % 64, j] = 1.0
    ind[64 + (j % 64), j] = 1.0
    return {
        "fmapc": fmap_core,
        "wt": wt,
        "relh": relh_slab.astype(bf),
        "relw": relw,
        "ind": ind.astype(bf),
        "onesh": np.ones((128, 128), np.float16),
        "bias4": np.full((128, 1), -4.0, np.float32),
    }


def _install_trace_hook():
    """Register the axon NTFF profiling hook (missing antenv.axon_hooks shim)
    and neuter the artifact upload so tracing works in this sandbox."""
    import sys
    import types
    import concourse.bass_utils as bu
    bu.upload_artifacts = lambda d: d
    try:
        from antenv import axon_hooks  # noqa: F401
        return
    except ImportError:
        pass
    import antenv
    mod = types.ModuleType("antenv.axon_hooks")
    mod._hook = None
    def set_axon_ntff_profile_hook(h):
        mod._hook = h
    def get_axon_ntff_profile_hook():
        return mod._hook
    mod.set_axon_ntff_profile_hook = set_axon_ntff_profile_hook
    mod.get_axon_ntff_profile_hook = get_axon_ntff_profile_hook
    sys.modules["antenv.axon_hooks"] = mod
    antenv.axon_hooks = mod
    try:
        from trn_agent_boot.trn_boot import _ntff_profile_via_ctypes
        h = _ntff_profile_via_ctypes("/opt/axon/libaxon_pjrt.so")
        if h is not None:
            mod._hook = h
    except Exception as e:
        print(f"trace hook install failed: {e}")


def kernel(fmap, w_qkv, rel_height, rel_width, _trace=False):
    global _GRAPH
    from concourse.bass_utils import run_bass_kernel_spmd

    fmap = np.asarray(fmap, dtype=np.float32)
    w_qkv = np.asarray(w_qkv, dtype=np.float32)
    rel_height = np.asarray(rel_height, dtype=np.float32)
    rel_width = np.asarray(rel_width, dtype=np.float32)

    if _GRAPH is None:
        _GRAPH = _build_graph()
    nc = _GRAPH

    in_maps = [_prep_core_inputs(fmap, w_qkv, rel_height, rel_width, c)
               for c in range(NCORES)]
    kw = {}
    if _trace:
        _install_trace_hook()
        import os
        os.makedirs("/tmp/ktrace", exist_ok=True)
        import tempfile
        kw = dict(tmpdir=tempfile.mkdtemp(dir="/tmp/ktrace"))
    res = run_bass_kernel_spmd(nc, in_maps, core_ids=list(range(NCORES)),
                               trace=_trace, **kw)
    out_full = np.zeros((C, L), np.float32)
    for c in range(NCORES):
        h, half = c // 2, c % 2
        out_full[h * 128:(h + 1) * 128, half * NQ:(half + 1) * NQ] = \
            np.asarray(res.results[c]["out"])
    if _trace:
        kernel._last_exec_time_ns = res.exec_time_ns
        kernel._last_profile = res.profile_json
    return out_full.reshape(1, C, H, W)


# revision 20
# speedup vs baseline: 1.1886x; 1.1886x over previous
"""Trainium2 Bass kernel for BotNet-style sparse attention (4 heads, 64x64 map,
dh=128, decomposed 2D relative position bias).

Sharding: 8 cores = 4 heads x 2 query-halves. Each core computes its head's
q/k/v from the full fmap, builds the rel-pos bias row tensors on chip, and runs
flash-style attention in "transposed sim" orientation (keys on partitions,
queries on free dim) so no attention-matrix transposes are needed:

  simT[k, q] = K^T.T @ Q^T  (+ bias via indicator-matmul accumulation)
  expT = exp(SCALE * simT - 4)           (ACT, PSUM->SBUF fp16)
  outT[d, q] = sum_k V[k, d] * expT[k,q] (PSUM accumulation over key chunks)
  rowsum via DVE accumulate + ones-matmul partition reduce
  out = outT * (1/rowsum) broadcast      (K=1 outer-product matmul broadcast)

The rel-pos bias decomposes per query q=(hq,wq), key k=(hk,wk) as
  bias = Rh[q, hk-hq+63] + Rw[q, wk-wq+63]
computed as 64-wide slices of rel^T against query groups (by image row for the
height term, by wq residue class for the width term), then folded into sim via
one extra accumulating matmul against a 0/1 indicator matrix.

Per-core inputs are key-permuted (own query half first) so the SPMD graph is
identical across cores; all per-core differences live in the input data.
"""

import numpy as np
import ml_dtypes

C, H, W = 512, 64, 64
HEADS, DH = 4, 128
L = H * W           # 4096
NQ = L // 2         # 2048 queries per core
QB = 1024           # query block
SCALE = DH ** -0.5
NCORES = 8

_GRAPH = None


def _build_graph():
    from concourse import bacc
    import concourse.mybir as mybir
    import concourse.tile as tile

    f32 = mybir.dt.float32
    bf16 = mybir.dt.bfloat16
    fp16 = mybir.dt.float16
    EXPF = mybir.ActivationFunctionType.Exp

    nc = bacc.Bacc(None)

    fmap_p = nc.declare_dram_parameter("fmapc", [C, L], bf16, isOutput=False)
    wt_p = nc.declare_dram_parameter("wt", [C, 384], bf16, isOutput=False)
    relh_p = nc.declare_dram_parameter("relh", [128, 96], bf16, isOutput=False)
    relw_p = nc.declare_dram_parameter("relw", [128, 127], bf16, isOutput=False)
    ind_p = nc.declare_dram_parameter("ind", [128, L], bf16, isOutput=False)
    onesh_p = nc.declare_dram_parameter("onesh", [128, 128], fp16, isOutput=False)
    bias4_p = nc.declare_dram_parameter("bias4", [128, 1], f32, isOutput=False)
    out_p = nc.declare_dram_parameter("out", [128, NQ], f32, isOutput=True)

    with tile.TileContext(nc) as tc:
        with tc.tile_pool(name="const", bufs=1) as cpool, \
             tc.tile_pool(name="big", bufs=1) as big, \
             tc.tile_pool(name="work", bufs=2) as work:

            # ---- constants to SBUF (small; after the gate DMAs) ----
            relh_sb = cpool.tile([128, 96], bf16, name="relh_sb")
            relw_sb = cpool.tile([128, 127], bf16, name="relw_sb")
            ind_sb = cpool.tile([128, L], bf16, name="ind_sb")
            onesh_sb = cpool.tile([128, 128], fp16, name="onesh_sb")
            bias4_sb = cpool.tile([128, 1], f32, name="bias4_sb")

            # ---- weights first (small, unblock qkv matmuls), then fmap
            # t-major so each 1024-column stripe completes across all four
            # c-tiles early; spread across engine DMA queues for bandwidth ----
            F4 = [big.tile([128, L], bf16, name=f"F{c}") for c in range(4)]
            W4 = []
            for c in range(4):
                w = big.tile([128, 384], bf16, name=f"W{c}")
                nc.scalar.dma_start(out=w, in_=wt_p[c * 128:(c + 1) * 128, :])
                W4.append(w)
            # bulk loads ride the two HWDGE queues (sync/scalar); gpsimd
            # SWDGE is too slow for bulk and only carries the tiny constants
            dma_engs = [nc.sync, nc.scalar, nc.scalar, nc.sync]
            for h in range(2):
                for c in range(4):
                    dma_engs[c].dma_start(
                        out=F4[c][:, h * 512:(h + 1) * 512],
                        in_=fmap_p[c * 128:(c + 1) * 128, h * 512:(h + 1) * 512])
            for c in range(4):
                dma_engs[c].dma_start(
                    out=F4[c][:, 1024:2048],
                    in_=fmap_p[c * 128:(c + 1) * 128, 1024:2048])
            nc.sync.dma_start(out=ind_sb[:, 0:1024], in_=ind_p[:, 0:1024])
            for t in range(2, 4):
                for c in range(4):
                    dma_engs[c].dma_start(
                        out=F4[c][:, t * 1024:(t + 1) * 1024],
                        in_=fmap_p[c * 128:(c + 1) * 128, t * 1024:(t + 1) * 1024])
            for k in range(1, 4):
                nc.sync.dma_start(out=ind_sb[:, k * 1024:(k + 1) * 1024],
                                  in_=ind_p[:, k * 1024:(k + 1) * 1024])
            nc.gpsimd.dma_start(out=relh_sb, in_=relh_p[:, :])
            nc.gpsimd.dma_start(out=relw_sb, in_=relw_p[:, :])
            nc.gpsimd.dma_start(out=onesh_sb, in_=onesh_p[:, :])
            nc.gpsimd.dma_start(out=bias4_sb, in_=bias4_p[:, :])

            QT = big.tile([128, NQ], bf16, name="QT")
            KT = big.tile([128, L], bf16, name="KT")
            VTt = big.tile([128, L], bf16, name="VTt")
            Vn = big.tile([128, L], bf16, name="Vn")
            BT = big.tile([128, NQ], bf16, name="BT")

            # ---- phase A+B: qkv projection pipelined with fmap stripe DMAs;
            # bias matmuls emitted mid-stream as PE gap-filler ----
            with tc.tile_pool(name="psA", bufs=2, space="PSUM") as psA:
                def qkv_group(dst, col0, t, eng):
                    ps = psA.tile([128, 1024], f32, name="qkv_ps", tag="qkv", bufs=2)
                    for c in range(4):
                        for h in range(2):
                            nc.tensor.matmul(
                                ps[:, h * 512:(h + 1) * 512],
                                W4[c][:, col0:col0 + 128],
                                F4[c][:, t * 1024 + h * 512: t * 1024 + (h + 1) * 512],
                                start=(c == 0), stop=(c == 3))
                    if eng == "act":
                        nc.scalar.copy(dst[:, t * 1024:(t + 1) * 1024], ps)
                    else:
                        nc.vector.tensor_copy(dst[:, t * 1024:(t + 1) * 1024], ps)

                def bias_half(h1):
                    q0 = h1 * 1024
                    bh_ps = psA.tile([64, 1024], f32, name="bh_ps", tag="bias", bufs=2)
                    for r in range(16):
                        rr = h1 * 16 + r
                        nc.tensor.matmul(
                            bh_ps[:, r * 64:(r + 1) * 64],
                            relh_sb[:, 31 - rr:95 - rr],
                            QT[:, q0 + r * 64:q0 + (r + 1) * 64],
                            start=True, stop=True)
                    nc.vector.tensor_copy(BT[0:64, q0:q0 + 1024], bh_ps)
                    bw_ps = psA.tile([64, 1024], f32, name="bw_ps", tag="bias", bufs=2)
                    for w in range(64):
                        nc.tensor.matmul(
                            bw_ps[:, w * 16:(w + 1) * 16],
                            relw_sb[:, 63 - w:127 - w],
                            QT.rearrange("d (i w) -> d w i", w=64)[:, w, h1 * 16:(h1 + 1) * 16],
                            start=True, stop=True)
                    nc.vector.tensor_copy(
                        BT[64:128, q0:q0 + 1024].rearrange("p (i w) -> p i w", i=16, w=64),
                        bw_ps.rearrange("p (w i) -> p i w", w=64, i=16))

                for t in range(4):
                    if t < 2:
                        qkv_group(QT, 0, t, "dve")
                    qkv_group(KT, 128, t, "act")
                    qkv_group(VTt, 256, t, "dve")
                    for s in range(t * 8, t * 8 + 8):
                        nc.sync.dma_start_transpose(
                            Vn[:, s * 128:(s + 1) * 128],
                            VTt[:, s * 128:(s + 1) * 128])
                    if t == 1:
                        bias_half(0)
                        bias_half(1)

            # ---- phase C: attention main loop ----
            with tc.tile_pool(name="psC", bufs=1, space="PSUM") as psC:
                for qb in range(2):
                    q0 = qb * QB
                    acc = work.tile([128, QB], fp16, name="acc", tag="acc", bufs=2)
                    outT = psC.tile([128, QB], f32, name="outT", tag="out", bufs=1)
                    for kc in range(32):
                        sim = psC.tile([128, QB], f32, name="sim", tag="sim", bufs=3)
                        for h in range(2):
                            sl = slice(q0 + h * 512, q0 + (h + 1) * 512)
                            po = sim[:, h * 512:(h + 1) * 512]
                            nc.tensor.matmul(
                                po, KT[:, kc * 128:(kc + 1) * 128], QT[:, sl],
                                start=True, stop=False)
                            nc.tensor.matmul(
                                po, ind_sb[:, kc * 128:(kc + 1) * 128], BT[:, sl],
                                start=False, stop=True)
                        expT = work.tile([128, QB], fp16, name="expT", tag="exp", bufs=4)
                        nc.scalar.activation(expT, sim, EXPF, bias=bias4_sb[:, 0:1], scale=SCALE)
                        if kc == 0:
                            nc.vector.tensor_copy(acc, expT)
                        else:
                            nc.vector.tensor_add(acc, acc, expT)
                        for h in range(2):
                            nc.tensor.matmul(
                                outT[:, h * 512:(h + 1) * 512],
                                Vn[:, kc * 128:(kc + 1) * 128],
                                expT[:, h * 512:(h + 1) * 512],
                                start=(kc == 0), stop=(kc == 31))

                    # rowsum: partition-reduce acc via ones-matmul
                    rs_ps = psC.tile([1, QB], f32, name="rs_ps", tag="sim", bufs=3)
                    for h in range(2):
                        nc.tensor.matmul(
                            rs_ps[:, h * 512:(h + 1) * 512],
                            onesh_sb[:, 0:1], acc[:, h * 512:(h + 1) * 512],
                            start=True, stop=True)
                    rs_row = work.tile([1, QB], fp16, name="rs_row", tag="rsrow")
                    nc.vector.tensor_copy(rs_row, rs_ps)
                    # broadcast rowsum across partitions (K=1 outer product),
                    # then wide approximate reciprocal and scale
                    bc_ps = psC.tile([128, QB], f32, name="bc_ps", tag="sim", bufs=3)
                    for hh in range(2):
                        nc.tensor.matmul(
                            bc_ps[:, hh * 512:(hh + 1) * 512],
                            onesh_sb[0:1, :], rs_row[0:1, hh * 512:(hh + 1) * 512],
                            start=True, stop=True)
                    rec_sb = work.tile([128, QB], f32, name="rec_sb", tag="bc")
                    nc.vector.reciprocal_approx_fast(out=rec_sb, in_=bc_ps)
                    out_sb = work.tile([128, QB], f32, name="out_sb", tag="osb")
                    nc.vector.tensor_mul(out_sb, outT, rec_sb)
                    nc.sync.dma_start(out=out_p[:, q0:q0 + QB], in_=out_sb)

    nc.finalize()
    return nc


def _prep_core_inputs(fmap, w_qkv, rel_height, rel_width, core):
    bf = ml_dtypes.bfloat16
    h, half = core // 2, core % 2
    q0 = half * NQ
    perm = (np.arange(L) + q0) % L
    fmap_flat = fmap.reshape(C, L)
    fmap_core = np.ascontiguousarray(fmap_flat[:, perm]).astype(bf)
    rows = np.r_[h * 128:(h + 1) * 128,
                 512 + h * 128:512 + (h + 1) * 128,
                 1024 + h * 128:1024 + (h + 1) * 128]
    wt = np.ascontiguousarray(w_qkv[rows].T).astype(bf)
    relhT = rel_height.T  # (128, 127)
    a = 32 * (1 - half)
    relh_slab = np.zeros((128, 96), np.float32)
    relh_slab[:, :95] = relhT[:, a:a + 95]
    relw = np.ascontiguousarray(rel_width.T).astype(bf)
    j = np.arange(L)
    ind = np.zeros((128, L), np.float32)
    ind[(j // 64 + 32 * half) % 64, j] = 1.0
    ind[64 + (j % 64), j] = 1.0
    return {
        "fmapc": fmap_core,
        "wt": wt,
        "relh": relh_slab.astype(bf),
        "relw": relw,
        "ind": ind.astype(bf),
        "onesh": np.ones((128, 128), np.float16),
        "bias4": np.full((128, 1), -4.0, np.float32),
    }


def _install_trace_hook():
    """Register the axon NTFF profiling hook (missing antenv.axon_hooks shim)
    and neuter the artifact upload so tracing works in this sandbox."""
    import sys
    import types
    import concourse.bass_utils as bu
    bu.upload_artifacts = lambda d: d
    try:
        from antenv import axon_hooks  # noqa: F401
        return
    except ImportError:
        pass
    import antenv
    mod = types.ModuleType("antenv.axon_hooks")
    mod._hook = None
    def set_axon_ntff_profile_hook(h):
        mod._hook = h
    def get_axon_ntff_profile_hook():
        return mod._hook
    mod.set_axon_ntff_profile_hook = set_axon_ntff_profile_hook
    mod.get_axon_ntff_profile_hook = get_axon_ntff_profile_hook
    sys.modules["antenv.axon_hooks"] = mod
    antenv.axon_hooks = mod
    try:
        from trn_agent_boot.trn_boot import _ntff_profile_via_ctypes
        h = _ntff_profile_via_ctypes("/opt/axon/libaxon_pjrt.so")
        if h is not None:
            mod._hook = h
    except Exception as e:
        print(f"trace hook install failed: {e}")


def kernel(fmap, w_qkv, rel_height, rel_width, _trace=False):
    global _GRAPH
    from concourse.bass_utils import run_bass_kernel_spmd

    fmap = np.asarray(fmap, dtype=np.float32)
    w_qkv = np.asarray(w_qkv, dtype=np.float32)
    rel_height = np.asarray(rel_height, dtype=np.float32)
    rel_width = np.asarray(rel_width, dtype=np.float32)

    if _GRAPH is None:
        _GRAPH = _build_graph()
    nc = _GRAPH

    in_maps = [_prep_core_inputs(fmap, w_qkv, rel_height, rel_width, c)
               for c in range(NCORES)]
    kw = {}
    if _trace:
        _install_trace_hook()
        import os
        os.makedirs("/tmp/ktrace", exist_ok=True)
        import tempfile
        kw = dict(tmpdir=tempfile.mkdtemp(dir="/tmp/ktrace"))
    res = run_bass_kernel_spmd(nc, in_maps, core_ids=list(range(NCORES)),
                               trace=_trace, **kw)
    out_full = np.zeros((C, L), np.float32)
    for c in range(NCORES):
        h, half = c // 2, c % 2
        out_full[h * 128:(h + 1) * 128, half * NQ:(half + 1) * NQ] = \
            np.asarray(res.results[c]["out"])
    if _trace:
        kernel._last_exec_time_ns = res.exec_time_ns
        kernel._last_profile = res.profile_json
    return out_full.reshape(1, C, H, W)


# revision 21
# speedup vs baseline: 1.1899x; 1.0011x over previous
"""Trainium2 Bass kernel for BotNet-style sparse attention (4 heads, 64x64 map,
dh=128, decomposed 2D relative position bias).

Sharding: 8 cores = 4 heads x 2 query-halves. Each core computes its head's
q/k/v from the full fmap, builds the rel-pos bias row tensors on chip, and runs
flash-style attention in "transposed sim" orientation (keys on partitions,
queries on free dim) so no attention-matrix transposes are needed:

  simT[k, q] = K^T.T @ Q^T  (+ bias via indicator-matmul accumulation)
  expT = exp(SCALE * simT - 4)           (ACT, PSUM->SBUF fp16)
  outT[d, q] = sum_k V[k, d] * expT[k,q] (PSUM accumulation over key chunks)
  rowsum via DVE accumulate + ones-matmul partition reduce
  out = outT * (1/rowsum) broadcast      (K=1 outer-product matmul broadcast)

The rel-pos bias decomposes per query q=(hq,wq), key k=(hk,wk) as
  bias = Rh[q, hk-hq+63] + Rw[q, wk-wq+63]
computed as 64-wide slices of rel^T against query groups (by image row for the
height term, by wq residue class for the width term), then folded into sim via
one extra accumulating matmul against a 0/1 indicator matrix.

Per-core inputs are key-permuted (own query half first) so the SPMD graph is
identical across cores; all per-core differences live in the input data.
"""

import numpy as np
import ml_dtypes

C, H, W = 512, 64, 64
HEADS, DH = 4, 128
L = H * W           # 4096
NQ = L // 2         # 2048 queries per core
QB = 1024           # query block
SCALE = DH ** -0.5
NCORES = 8

_GRAPH = None


def _build_graph():
    from concourse import bacc
    import concourse.mybir as mybir
    import concourse.tile as tile

    f32 = mybir.dt.float32
    bf16 = mybir.dt.bfloat16
    fp16 = mybir.dt.float16
    EXPF = mybir.ActivationFunctionType.Exp

    nc = bacc.Bacc(None)

    fmap_p = nc.declare_dram_parameter("fmapc", [C, L], bf16, isOutput=False)
    wt_p = nc.declare_dram_parameter("wt", [C, 384], bf16, isOutput=False)
    relh_p = nc.declare_dram_parameter("relh", [128, 96], bf16, isOutput=False)
    relw_p = nc.declare_dram_parameter("relw", [128, 127], bf16, isOutput=False)
    ind_p = nc.declare_dram_parameter("ind", [128, L], bf16, isOutput=False)
    onesh_p = nc.declare_dram_parameter("onesh", [128, 128], fp16, isOutput=False)
    bias4_p = nc.declare_dram_parameter("bias4", [128, 1], f32, isOutput=False)
    out_p = nc.declare_dram_parameter("out", [128, NQ], f32, isOutput=True)

    with tile.TileContext(nc) as tc:
        with tc.tile_pool(name="const", bufs=1) as cpool, \
             tc.tile_pool(name="big", bufs=1) as big, \
             tc.tile_pool(name="work", bufs=2) as work:

            # ---- constants to SBUF (small; after the gate DMAs) ----
            relh_sb = cpool.tile([128, 96], bf16, name="relh_sb")
            relw_sb = cpool.tile([128, 127], bf16, name="relw_sb")
            ind_sb = cpool.tile([128, L], bf16, name="ind_sb")
            onesh_sb = cpool.tile([128, 128], fp16, name="onesh_sb")
            bias4_sb = cpool.tile([128, 1], f32, name="bias4_sb")

            # ---- weights first (small, unblock qkv matmuls), then fmap
            # t-major so each 1024-column stripe completes across all four
            # c-tiles early; spread across engine DMA queues for bandwidth ----
            F4 = [big.tile([128, L], bf16, name=f"F{c}") for c in range(4)]
            W4 = []
            for c in range(4):
                w = big.tile([128, 384], bf16, name=f"W{c}")
                nc.scalar.dma_start(out=w, in_=wt_p[c * 128:(c + 1) * 128, :])
                W4.append(w)
            # bulk loads ride the two HWDGE queues (sync/scalar); gpsimd
            # SWDGE is too slow for bulk and only carries the tiny constants
            dma_engs = [nc.sync, nc.scalar, nc.scalar, nc.sync]
            for h in range(2):
                for c in range(4):
                    dma_engs[c].dma_start(
                        out=F4[c][:, h * 512:(h + 1) * 512],
                        in_=fmap_p[c * 128:(c + 1) * 128, h * 512:(h + 1) * 512])
            for c in range(4):
                dma_engs[c].dma_start(
                    out=F4[c][:, 1024:2048],
                    in_=fmap_p[c * 128:(c + 1) * 128, 1024:2048])
            nc.sync.dma_start(out=ind_sb[:, 0:1024], in_=ind_p[:, 0:1024])
            for t in range(2, 4):
                for c in range(4):
                    dma_engs[c].dma_start(
                        out=F4[c][:, t * 1024:(t + 1) * 1024],
                        in_=fmap_p[c * 128:(c + 1) * 128, t * 1024:(t + 1) * 1024])
            for k in range(1, 4):
                nc.sync.dma_start(out=ind_sb[:, k * 1024:(k + 1) * 1024],
                                  in_=ind_p[:, k * 1024:(k + 1) * 1024])
            nc.gpsimd.dma_start(out=relh_sb, in_=relh_p[:, :])
            nc.gpsimd.dma_start(out=relw_sb, in_=relw_p[:, :])
            nc.gpsimd.dma_start(out=onesh_sb, in_=onesh_p[:, :])
            nc.gpsimd.dma_start(out=bias4_sb, in_=bias4_p[:, :])

            QT = big.tile([128, NQ], bf16, name="QT")
            KT = big.tile([128, L], bf16, name="KT")
            VTt = big.tile([128, L], bf16, name="VTt")
            Vn = big.tile([128, L], bf16, name="Vn")
            BT = big.tile([128, NQ], bf16, name="BT")

            # ---- phase A+B: qkv projection pipelined with fmap stripe DMAs;
            # bias matmuls emitted mid-stream as PE gap-filler ----
            with tc.tile_pool(name="psA", bufs=2, space="PSUM") as psA:
                def qkv_group(dst, col0, t, eng):
                    ps = psA.tile([128, 1024], f32, name="qkv_ps", tag="qkv", bufs=2)
                    for c in range(4):
                        for h in range(2):
                            nc.tensor.matmul(
                                ps[:, h * 512:(h + 1) * 512],
                                W4[c][:, col0:col0 + 128],
                                F4[c][:, t * 1024 + h * 512: t * 1024 + (h + 1) * 512],
                                start=(c == 0), stop=(c == 3))
                    if eng == "act":
                        nc.scalar.copy(dst[:, t * 1024:(t + 1) * 1024], ps)
                    else:
                        nc.vector.tensor_copy(dst[:, t * 1024:(t + 1) * 1024], ps)

                def bias_all():
                    bh_ps = psA.tile([64, NQ], f32, name="bh_ps", tag="bias", bufs=1)
                    for r in range(32):
                        nc.tensor.matmul(
                            bh_ps[:, r * 64:(r + 1) * 64],
                            relh_sb[:, 31 - r:95 - r],
                            QT[:, r * 64:(r + 1) * 64],
                            start=True, stop=True)
                    nc.vector.tensor_copy(BT[0:64, :], bh_ps)
                    bw_ps = psA.tile([64, NQ], f32, name="bw_ps", tag="bias", bufs=1)
                    for w in range(64):
                        nc.tensor.matmul(
                            bw_ps[:, w * 32:(w + 1) * 32],
                            relw_sb[:, 63 - w:127 - w],
                            QT.rearrange("d (i w) -> d w i", w=64)[:, w, :],
                            start=True, stop=True)
                    nc.vector.tensor_copy(
                        BT[64:128, :].rearrange("p (i w) -> p i w", i=32, w=64),
                        bw_ps.rearrange("p (w i) -> p i w", w=64, i=32))

                for t in range(4):
                    if t < 2:
                        qkv_group(QT, 0, t, "dve")
                    qkv_group(KT, 128, t, "act")
                    qkv_group(VTt, 256, t, "dve")
                    for s in range(t * 8, t * 8 + 8):
                        nc.sync.dma_start_transpose(
                            Vn[:, s * 128:(s + 1) * 128],
                            VTt[:, s * 128:(s + 1) * 128])
                    if t == 1:
                        bias_all()

            # ---- phase C: attention main loop ----
            with tc.tile_pool(name="psC", bufs=1, space="PSUM") as psC:
                for qb in range(2):
                    q0 = qb * QB
                    acc = work.tile([128, QB], fp16, name="acc", tag="acc", bufs=2)
                    outT = psC.tile([128, QB], f32, name="outT", tag="out", bufs=1)
                    for kc in range(32):
                        sim = psC.tile([128, QB], f32, name="sim", tag="sim", bufs=3)
                        for h in range(2):
                            sl = slice(q0 + h * 512, q0 + (h + 1) * 512)
                            po = sim[:, h * 512:(h + 1) * 512]
                            nc.tensor.matmul(
                                po, KT[:, kc * 128:(kc + 1) * 128], QT[:, sl],
                                start=True, stop=False)
                            nc.tensor.matmul(
                                po, ind_sb[:, kc * 128:(kc + 1) * 128], BT[:, sl],
                                start=False, stop=True)
                        expT = work.tile([128, QB], fp16, name="expT", tag="exp", bufs=4)
                        nc.scalar.activation(expT, sim, EXPF, bias=bias4_sb[:, 0:1], scale=SCALE)
                        if kc == 0:
                            nc.vector.tensor_copy(acc, expT)
                        else:
                            nc.vector.tensor_add(acc, acc, expT)
                        for h in range(2):
                            nc.tensor.matmul(
                                outT[:, h * 512:(h + 1) * 512],
                                Vn[:, kc * 128:(kc + 1) * 128],
                                expT[:, h * 512:(h + 1) * 512],
                                start=(kc == 0), stop=(kc == 31))

                    # rowsum: partition-reduce acc via ones-matmul
                    rs_ps = psC.tile([1, QB], f32, name="rs_ps", tag="sim", bufs=3)
                    for h in range(2):
                        nc.tensor.matmul(
                            rs_ps[:, h * 512:(h + 1) * 512],
                            onesh_sb[:, 0:1], acc[:, h * 512:(h + 1) * 512],
                            start=True, stop=True)
                    rs_row = work.tile([1, QB], fp16, name="rs_row", tag="rsrow")
                    nc.vector.tensor_copy(rs_row, rs_ps)
                    # broadcast rowsum across partitions (K=1 outer product),
                    # then wide approximate reciprocal and scale
                    bc_ps = psC.tile([128, QB], f32, name="bc_ps", tag="sim", bufs=3)
                    for hh in range(2):
                        nc.tensor.matmul(
                            bc_ps[:, hh * 512:(hh + 1) * 512],
                            onesh_sb[0:1, :], rs_row[0:1, hh * 512:(hh + 1) * 512],
                            start=True, stop=True)
                    rec_sb = work.tile([128, QB], f32, name="rec_sb", tag="bc")
                    nc.vector.reciprocal_approx_fast(out=rec_sb, in_=bc_ps)
                    out_sb = work.tile([128, QB], f32, name="out_sb", tag="osb")
                    nc.vector.tensor_mul(out_sb, outT, rec_sb)
                    nc.sync.dma_start(out=out_p[:, q0:q0 + QB], in_=out_sb)

    nc.finalize()
    return nc


def _prep_core_inputs(fmap, w_qkv, rel_height, rel_width, core):
    bf = ml_dtypes.bfloat16
    h, half = core // 2, core % 2
    q0 = half * NQ
    perm = (np.arange(L) + q0) % L
    fmap_flat = fmap.reshape(C, L)
    fmap_core = np.ascontiguousarray(fmap_flat[:, perm]).astype(bf)
    rows = np.r_[h * 128:(h + 1) * 128,
                 512 + h * 128:512 + (h + 1) * 128,
                 1024 + h * 128:1024 + (h + 1) * 128]
    wt = np.ascontiguousarray(w_qkv[rows].T).astype(bf)
    relhT = rel_height.T  # (128, 127)
    a = 32 * (1 - half)
    relh_slab = np.zeros((128, 96), np.float32)
    relh_slab[:, :95] = relhT[:, a:a + 95]
    relw = np.ascontiguousarray(rel_width.T).astype(bf)
    j = np.arange(L)
    ind = np.zeros((128, L), np.float32)
    ind[(j // 64 + 32 * half) % 64, j] = 1.0
    ind[64 + (j % 64), j] = 1.0
    return {
        "fmapc": fmap_core,
        "wt": wt,
        "relh": relh_slab.astype(bf),
        "relw": relw,
        "ind": ind.astype(bf),
        "onesh": np.ones((128, 128), np.float16),
        "bias4": np.full((128, 1), -4.0, np.float32),
    }


def _install_trace_hook():
    """Register the axon NTFF profiling hook (missing antenv.axon_hooks shim)
    and neuter the artifact upload so tracing works in this sandbox."""
    import sys
    import types
    import concourse.bass_utils as bu
    bu.upload_artifacts = lambda d: d
    try:
        from antenv import axon_hooks  # noqa: F401
        return
    except ImportError:
        pass
    import antenv
    mod = types.ModuleType("antenv.axon_hooks")
    mod._hook = None
    def set_axon_ntff_profile_hook(h):
        mod._hook = h
    def get_axon_ntff_profile_hook():
        return mod._hook
    mod.set_axon_ntff_profile_hook = set_axon_ntff_profile_hook
    mod.get_axon_ntff_profile_hook = get_axon_ntff_profile_hook
    sys.modules["antenv.axon_hooks"] = mod
    antenv.axon_hooks = mod
    try:
        from trn_agent_boot.trn_boot import _ntff_profile_via_ctypes
        h = _ntff_profile_via_ctypes("/opt/axon/libaxon_pjrt.so")
        if h is not None:
            mod._hook = h
    except Exception as e:
        print(f"trace hook install failed: {e}")


def kernel(fmap, w_qkv, rel_height, rel_width, _trace=False):
    global _GRAPH
    from concourse.bass_utils import run_bass_kernel_spmd

    fmap = np.asarray(fmap, dtype=np.float32)
    w_qkv = np.asarray(w_qkv, dtype=np.float32)
    rel_height = np.asarray(rel_height, dtype=np.float32)
    rel_width = np.asarray(rel_width, dtype=np.float32)

    if _GRAPH is None:
        _GRAPH = _build_graph()
    nc = _GRAPH

    in_maps = [_prep_core_inputs(fmap, w_qkv, rel_height, rel_width, c)
               for c in range(NCORES)]
    kw = {}
    if _trace:
        _install_trace_hook()
        import os
        os.makedirs("/tmp/ktrace", exist_ok=True)
        import tempfile
        kw = dict(tmpdir=tempfile.mkdtemp(dir="/tmp/ktrace"))
    res = run_bass_kernel_spmd(nc, in_maps, core_ids=list(range(NCORES)),
                               trace=_trace, **kw)
    out_full = np.zeros((C, L), np.float32)
    for c in range(NCORES):
        h, half = c // 2, c % 2
        out_full[h * 128:(h + 1) * 128, half * NQ:(half + 1) * NQ] = \
            np.asarray(res.results[c]["out"])
    if _trace:
        kernel._last_exec_time_ns = res.exec_time_ns
        kernel._last_profile = res.profile_json
    return out_full.reshape(1, C, H, W)


# revision 22
# speedup vs baseline: 1.1915x; 1.0014x over previous
"""Trainium2 Bass kernel for BotNet-style sparse attention (4 heads, 64x64 map,
dh=128, decomposed 2D relative position bias).

Sharding: 8 cores = 4 heads x 2 query-halves. Each core computes its head's
q/k/v from the full fmap, builds the rel-pos bias row tensors on chip, and runs
flash-style attention in "transposed sim" orientation (keys on partitions,
queries on free dim) so no attention-matrix transposes are needed:

  simT[k, q] = K^T.T @ Q^T  (+ bias via indicator-matmul accumulation)
  expT = exp(SCALE * simT - 4)           (ACT, PSUM->SBUF fp16)
  outT[d, q] = sum_k V[k, d] * expT[k,q] (PSUM accumulation over key chunks)
  rowsum via DVE accumulate + ones-matmul partition reduce
  out = outT * (1/rowsum) broadcast      (K=1 outer-product matmul broadcast)

The rel-pos bias decomposes per query q=(hq,wq), key k=(hk,wk) as
  bias = Rh[q, hk-hq+63] + Rw[q, wk-wq+63]
computed as 64-wide slices of rel^T against query groups (by image row for the
height term, by wq residue class for the width term), then folded into sim via
one extra accumulating matmul against a 0/1 indicator matrix.

Per-core inputs are key-permuted (own query half first) so the SPMD graph is
identical across cores; all per-core differences live in the input data.
"""

import numpy as np
import ml_dtypes

C, H, W = 512, 64, 64
HEADS, DH = 4, 128
L = H * W           # 4096
NQ = L // 2         # 2048 queries per core
QB = 1024           # query block
SCALE = DH ** -0.5
NCORES = 8

_GRAPH = None


def _build_graph():
    from concourse import bacc
    import concourse.mybir as mybir
    import concourse.tile as tile

    f32 = mybir.dt.float32
    bf16 = mybir.dt.bfloat16
    fp16 = mybir.dt.float16
    EXPF = mybir.ActivationFunctionType.Exp

    nc = bacc.Bacc(None)

    fmap_p = nc.declare_dram_parameter("fmapc", [C, L], bf16, isOutput=False)
    wt_p = nc.declare_dram_parameter("wt", [C, 384], bf16, isOutput=False)
    relh_p = nc.declare_dram_parameter("relh", [128, 96], bf16, isOutput=False)
    relw_p = nc.declare_dram_parameter("relw", [128, 127], bf16, isOutput=False)
    ind_p = nc.declare_dram_parameter("ind", [128, L], bf16, isOutput=False)
    onesh_p = nc.declare_dram_parameter("onesh", [128, 128], fp16, isOutput=False)
    bias4_p = nc.declare_dram_parameter("bias4", [128, 1], f32, isOutput=False)
    out_p = nc.declare_dram_parameter("out", [128, NQ], f32, isOutput=True)

    with tile.TileContext(nc) as tc:
        with tc.tile_pool(name="const", bufs=1) as cpool, \
             tc.tile_pool(name="big", bufs=1) as big, \
             tc.tile_pool(name="work", bufs=2) as work:

            # ---- constants to SBUF (small; after the gate DMAs) ----
            relh_sb = cpool.tile([128, 96], bf16, name="relh_sb")
            relw_sb = cpool.tile([128, 127], bf16, name="relw_sb")
            ind_sb = cpool.tile([128, L], bf16, name="ind_sb")
            onesh_sb = cpool.tile([128, 128], fp16, name="onesh_sb")
            bias4_sb = cpool.tile([128, 1], f32, name="bias4_sb")

            # ---- weights first (small, unblock qkv matmuls), then fmap
            # t-major so each 1024-column stripe completes across all four
            # c-tiles early; spread across engine DMA queues for bandwidth ----
            F4 = [big.tile([128, L], bf16, name=f"F{c}") for c in range(4)]
            W4 = []
            for c in range(4):
                w = big.tile([128, 384], bf16, name=f"W{c}")
                nc.scalar.dma_start(out=w, in_=wt_p[c * 128:(c + 1) * 128, :])
                W4.append(w)
            # bulk loads ride the two HWDGE queues (sync/scalar); gpsimd
            # SWDGE is too slow for bulk and only carries the tiny constants
            dma_engs = [nc.sync, nc.scalar, nc.scalar, nc.sync]
            for h in range(2):
                for c in range(4):
                    dma_engs[c].dma_start(
                        out=F4[c][:, h * 512:(h + 1) * 512],
                        in_=fmap_p[c * 128:(c + 1) * 128, h * 512:(h + 1) * 512])
            for c in range(4):
                dma_engs[c].dma_start(
                    out=F4[c][:, 1024:2048],
                    in_=fmap_p[c * 128:(c + 1) * 128, 1024:2048])
            nc.sync.dma_start(out=ind_sb[:, 0:1024], in_=ind_p[:, 0:1024])
            for t in range(2, 4):
                for c in range(4):
                    dma_engs[c].dma_start(
                        out=F4[c][:, t * 1024:(t + 1) * 1024],
                        in_=fmap_p[c * 128:(c + 1) * 128, t * 1024:(t + 1) * 1024])
            for k in range(1, 4):
                nc.sync.dma_start(out=ind_sb[:, k * 1024:(k + 1) * 1024],
                                  in_=ind_p[:, k * 1024:(k + 1) * 1024])
            nc.gpsimd.dma_start(out=relh_sb, in_=relh_p[:, :])
            nc.gpsimd.dma_start(out=relw_sb, in_=relw_p[:, :])
            nc.gpsimd.dma_start(out=onesh_sb, in_=onesh_p[:, :])
            nc.gpsimd.dma_start(out=bias4_sb, in_=bias4_p[:, :])

            QT = big.tile([128, NQ], bf16, name="QT")
            KT = big.tile([128, L], bf16, name="KT")
            VTt = big.tile([128, L], bf16, name="VTt")
            Vn = big.tile([128, L], bf16, name="Vn")
            BT = big.tile([128, NQ], bf16, name="BT")

            # ---- PE warmup: ~4us of dummy matmuls on a memset tile so the
            # HAM clock-gate opens before the first real matmul arrives ----
            warm_sb = work.tile([128, 512], bf16, name="warm_sb", tag="warm")
            nc.gpsimd.memset(warm_sb, 0.0)
            with tc.tile_pool(name="psW", bufs=1, space="PSUM") as psW:
                wps = psW.tile([128, 512], f32, name="warm_ps", tag="warm")
                for _ in range(12):
                    nc.tensor.matmul(wps, warm_sb[:, 0:128], warm_sb,
                                     start=True, stop=True)

            # ---- phase A+B: qkv projection pipelined with fmap stripe DMAs;
            # bias matmuls emitted mid-stream as PE gap-filler ----
            with tc.tile_pool(name="psA", bufs=2, space="PSUM") as psA:
                def qkv_group(dst, col0, t, eng):
                    ps = psA.tile([128, 1024], f32, name="qkv_ps", tag="qkv", bufs=2)
                    for c in range(4):
                        for h in range(2):
                            nc.tensor.matmul(
                                ps[:, h * 512:(h + 1) * 512],
                                W4[c][:, col0:col0 + 128],
                                F4[c][:, t * 1024 + h * 512: t * 1024 + (h + 1) * 512],
                                start=(c == 0), stop=(c == 3))
                    if eng == "act":
                        nc.scalar.copy(dst[:, t * 1024:(t + 1) * 1024], ps)
                    else:
                        nc.vector.tensor_copy(dst[:, t * 1024:(t + 1) * 1024], ps)

                def bias_all():
                    bh_ps = psA.tile([64, NQ], f32, name="bh_ps", tag="bias", bufs=1)
                    for r in range(32):
                        nc.tensor.matmul(
                            bh_ps[:, r * 64:(r + 1) * 64],
                            relh_sb[:, 31 - r:95 - r],
                            QT[:, r * 64:(r + 1) * 64],
                            start=True, stop=True)
                    nc.vector.tensor_copy(BT[0:64, :], bh_ps)
                    bw_ps = psA.tile([64, NQ], f32, name="bw_ps", tag="bias", bufs=1)
                    for w in range(64):
                        nc.tensor.matmul(
                            bw_ps[:, w * 32:(w + 1) * 32],
                            relw_sb[:, 63 - w:127 - w],
                            QT.rearrange("d (i w) -> d w i", w=64)[:, w, :],
                            start=True, stop=True)
                    nc.vector.tensor_copy(
                        BT[64:128, :].rearrange("p (i w) -> p i w", i=32, w=64),
                        bw_ps.rearrange("p (w i) -> p i w", w=64, i=32))

                for t in range(4):
                    if t < 2:
                        qkv_group(QT, 0, t, "dve")
                    qkv_group(KT, 128, t, "act")
                    qkv_group(VTt, 256, t, "dve")
                    for s in range(t * 8, t * 8 + 8):
                        nc.sync.dma_start_transpose(
                            Vn[:, s * 128:(s + 1) * 128],
                            VTt[:, s * 128:(s + 1) * 128])
                    if t == 1:
                        bias_all()

            # ---- phase C: attention main loop ----
            with tc.tile_pool(name="psC", bufs=1, space="PSUM") as psC:
                for qb in range(2):
                    q0 = qb * QB
                    acc = work.tile([128, QB], fp16, name="acc", tag="acc", bufs=2)
                    outT = psC.tile([128, QB], f32, name="outT", tag="out", bufs=1)
                    for kc in range(32):
                        sim = psC.tile([128, QB], f32, name="sim", tag="sim", bufs=3)
                        for h in range(2):
                            sl = slice(q0 + h * 512, q0 + (h + 1) * 512)
                            po = sim[:, h * 512:(h + 1) * 512]
                            nc.tensor.matmul(
                                po, KT[:, kc * 128:(kc + 1) * 128], QT[:, sl],
                                start=True, stop=False)
                            nc.tensor.matmul(
                                po, ind_sb[:, kc * 128:(kc + 1) * 128], BT[:, sl],
                                start=False, stop=True)
                        expT = work.tile([128, QB], fp16, name="expT", tag="exp", bufs=4)
                        nc.scalar.activation(expT, sim, EXPF, bias=bias4_sb[:, 0:1], scale=SCALE)
                        if kc == 0:
                            nc.vector.tensor_copy(acc, expT)
                        else:
                            nc.vector.tensor_add(acc, acc, expT)
                        for h in range(2):
                            nc.tensor.matmul(
                                outT[:, h * 512:(h + 1) * 512],
                                Vn[:, kc * 128:(kc + 1) * 128],
                                expT[:, h * 512:(h + 1) * 512],
                                start=(kc == 0), stop=(kc == 31))

                    # rowsum: partition-reduce acc via ones-matmul
                    rs_ps = psC.tile([1, QB], f32, name="rs_ps", tag="sim", bufs=3)
                    for h in range(2):
                        nc.tensor.matmul(
                            rs_ps[:, h * 512:(h + 1) * 512],
                            onesh_sb[:, 0:1], acc[:, h * 512:(h + 1) * 512],
                            start=True, stop=True)
                    rs_row = work.tile([1, QB], fp16, name="rs_row", tag="rsrow")
                    nc.vector.tensor_copy(rs_row, rs_ps)
                    # broadcast rowsum across partitions (K=1 outer product),
                    # then wide approximate reciprocal and scale
                    bc_ps = psC.tile([128, QB], f32, name="bc_ps", tag="sim", bufs=3)
                    for hh in range(2):
                        nc.tensor.matmul(
                            bc_ps[:, hh * 512:(hh + 1) * 512],
                            onesh_sb[0:1, :], rs_row[0:1, hh * 512:(hh + 1) * 512],
                            start=True, stop=True)
                    rec_sb = work.tile([128, QB], f32, name="rec_sb", tag="bc")
                    nc.vector.reciprocal_approx_fast(out=rec_sb, in_=bc_ps)
                    out_sb = work.tile([128, QB], f32, name="out_sb", tag="osb")
                    nc.vector.tensor_mul(out_sb, outT, rec_sb)
                    nc.sync.dma_start(out=out_p[:, q0:q0 + QB], in_=out_sb)

    nc.finalize()
    return nc


def _prep_core_inputs(fmap, w_qkv, rel_height, rel_width, core):
    bf = ml_dtypes.bfloat16
    h, half = core // 2, core % 2
    q0 = half * NQ
    perm = (np.arange(L) + q0) % L
    fmap_flat = fmap.reshape(C, L)
    fmap_core = np.ascontiguousarray(fmap_flat[:, perm]).astype(bf)
    rows = np.r_[h * 128:(h + 1) * 128,
                 512 + h * 128:512 + (h + 1) * 128,
                 1024 + h * 128:1024 + (h + 1) * 128]
    wt = np.ascontiguousarray(w_qkv[rows].T).astype(bf)
    relhT = rel_height.T  # (128, 127)
    a = 32 * (1 - half)
    relh_slab = np.zeros((128, 96), np.float32)
    relh_slab[:, :95] = relhT[:, a:a + 95]
    relw = np.ascontiguousarray(rel_width.T).astype(bf)
    j = np.arange(L)
    ind = np.zeros((128, L), np.float32)
    ind[(j // 64 + 32 * half) % 64, j] = 1.0
    ind[64 + (j % 64), j] = 1.0
    return {
        "fmapc": fmap_core,
        "wt": wt,
        "relh": relh_slab.astype(bf),
        "relw": relw,
        "ind": ind.astype(bf),
        "onesh": np.ones((128, 128), np.float16),
        "bias4": np.full((128, 1), -4.0, np.float32),
    }


def _install_trace_hook():
    """Register the axon NTFF profiling hook (missing antenv.axon_hooks shim)
    and neuter the artifact upload so tracing works in this sandbox."""
    import sys
    import types
    import concourse.bass_utils as bu
    bu.upload_artifacts = lambda d: d
    try:
        from antenv import axon_hooks  # noqa: F401
        return
    except ImportError:
        pass
    import antenv
    mod = types.ModuleType("antenv.axon_hooks")
    mod._hook = None
    def set_axon_ntff_profile_hook(h):
        mod._hook = h
    def get_axon_ntff_profile_hook():
        return mod._hook
    mod.set_axon_ntff_profile_hook = set_axon_ntff_profile_hook
    mod.get_axon_ntff_profile_hook = get_axon_ntff_profile_hook
    sys.modules["antenv.axon_hooks"] = mod
    antenv.axon_hooks = mod
    try:
        from trn_agent_boot.trn_boot import _ntff_profile_via_ctypes
        h = _ntff_profile_via_ctypes("/opt/axon/libaxon_pjrt.so")
        if h is not None:
            mod._hook = h
    except Exception as e:
        print(f"trace hook install failed: {e}")


def kernel(fmap, w_qkv, rel_height, rel_width, _trace=False):
    global _GRAPH
    from concourse.bass_utils import run_bass_kernel_spmd

    fmap = np.asarray(fmap, dtype=np.float32)
    w_qkv = np.asarray(w_qkv, dtype=np.float32)
    rel_height = np.asarray(rel_height, dtype=np.float32)
    rel_width = np.asarray(rel_width, dtype=np.float32)

    if _GRAPH is None:
        _GRAPH = _build_graph()
    nc = _GRAPH

    in_maps = [_prep_core_inputs(fmap, w_qkv, rel_height, rel_width, c)
               for c in range(NCORES)]
    kw = {}
    if _trace:
        _install_trace_hook()
        import os
        os.makedirs("/tmp/ktrace", exist_ok=True)
        import tempfile
        kw = dict(tmpdir=tempfile.mkdtemp(dir="/tmp/ktrace"))
    res = run_bass_kernel_spmd(nc, in_maps, core_ids=list(range(NCORES)),
                               trace=_trace, **kw)
    out_full = np.zeros((C, L), np.float32)
    for c in range(NCORES):
        h, half = c // 2, c % 2
        out_full[h * 128:(h + 1) * 128, half * NQ:(half + 1) * NQ] = \
            np.asarray(res.results[c]["out"])
    if _trace:
        kernel._last_exec_time_ns = res.exec_time_ns
        kernel._last_profile = res.profile_json
    return out_full.reshape(1, C, H, W)


# revision 23
# speedup vs baseline: 1.2000x; 1.0071x over previous
"""Trainium2 Bass kernel for BotNet-style sparse attention (4 heads, 64x64 map,
dh=128, decomposed 2D relative position bias).

Sharding: 8 cores = 4 heads x 2 query-halves. Each core computes its head's
q/k/v from the full fmap, builds the rel-pos bias row tensors on chip, and runs
flash-style attention in "transposed sim" orientation (keys on partitions,
queries on free dim) so no attention-matrix transposes are needed:

  simT[k, q] = K^T.T @ Q^T  (+ bias via indicator-matmul accumulation)
  expT = exp(SCALE * simT - 4)           (ACT, PSUM->SBUF fp16)
  outT[d, q] = sum_k V[k, d] * expT[k,q] (PSUM accumulation over key chunks)
  rowsum via DVE accumulate + ones-matmul partition reduce
  out = outT * (1/rowsum) broadcast      (K=1 outer-product matmul broadcast)

The rel-pos bias decomposes per query q=(hq,wq), key k=(hk,wk) as
  bias = Rh[q, hk-hq+63] + Rw[q, wk-wq+63]
computed as 64-wide slices of rel^T against query groups (by image row for the
height term, by wq residue class for the width term), then folded into sim via
one extra accumulating matmul against a 0/1 indicator matrix.

Per-core inputs are key-permuted (own query half first) so the SPMD graph is
identical across cores; all per-core differences live in the input data.
"""

import numpy as np
import ml_dtypes

C, H, W = 512, 64, 64
HEADS, DH = 4, 128
L = H * W           # 4096
NQ = L // 2         # 2048 queries per core
QB = 1024           # query block
SCALE = DH ** -0.5
NCORES = 8

_GRAPH = None


def _build_graph():
    from concourse import bacc
    import concourse.mybir as mybir
    import concourse.tile as tile

    f32 = mybir.dt.float32
    bf16 = mybir.dt.bfloat16
    fp16 = mybir.dt.float16
    EXPF = mybir.ActivationFunctionType.Exp

    nc = bacc.Bacc(None)

    fmap_p = nc.declare_dram_parameter("fmapc", [C, L], bf16, isOutput=False)
    wt_p = nc.declare_dram_parameter("wt", [C, 384], bf16, isOutput=False)
    relh_p = nc.declare_dram_parameter("relh", [128, 96], bf16, isOutput=False)
    relw_p = nc.declare_dram_parameter("relw", [128, 127], bf16, isOutput=False)
    ind_p = nc.declare_dram_parameter("ind", [128, L], bf16, isOutput=False)
    onesh_p = nc.declare_dram_parameter("onesh", [128, 128], fp16, isOutput=False)
    bias4_p = nc.declare_dram_parameter("bias4", [128, 1], f32, isOutput=False)
    out_p = nc.declare_dram_parameter("out", [128, NQ], f32, isOutput=True)

    with tile.TileContext(nc) as tc:
        with tc.tile_pool(name="const", bufs=1) as cpool, \
             tc.tile_pool(name="big", bufs=1) as big, \
             tc.tile_pool(name="work", bufs=2) as work:

            # ---- constants to SBUF (small; after the gate DMAs) ----
            relh_sb = cpool.tile([128, 96], bf16, name="relh_sb")
            relw_sb = cpool.tile([128, 127], bf16, name="relw_sb")
            ind_sb = cpool.tile([128, L], bf16, name="ind_sb")
            onesh_sb = cpool.tile([128, 128], fp16, name="onesh_sb")
            bias4_sb = cpool.tile([128, 1], f32, name="bias4_sb")

            # ---- weights first (small, unblock qkv matmuls), then fmap
            # t-major so each 1024-column stripe completes across all four
            # c-tiles early; spread across engine DMA queues for bandwidth ----
            F4 = [big.tile([128, L], bf16, name=f"F{c}") for c in range(4)]
            W4 = []
            for c in range(4):
                w = big.tile([128, 384], bf16, name=f"W{c}")
                nc.scalar.dma_start(out=w, in_=wt_p[c * 128:(c + 1) * 128, :])
                W4.append(w)
            # bulk loads ride the two HWDGE queues (sync/scalar); gpsimd
            # SWDGE is too slow for bulk and only carries the tiny constants
            dma_engs = [nc.sync, nc.scalar, nc.scalar, nc.sync]
            for h in range(2):
                for c in range(4):
                    dma_engs[c].dma_start(
                        out=F4[c][:, h * 512:(h + 1) * 512],
                        in_=fmap_p[c * 128:(c + 1) * 128, h * 512:(h + 1) * 512])
            for c in range(4):
                dma_engs[c].dma_start(
                    out=F4[c][:, 1024:2048],
                    in_=fmap_p[c * 128:(c + 1) * 128, 1024:2048])
            nc.sync.dma_start(out=ind_sb[:, 0:1024], in_=ind_p[:, 0:1024])
            for t in range(2, 4):
                for c in range(4):
                    dma_engs[c].dma_start(
                        out=F4[c][:, t * 1024:(t + 1) * 1024],
                        in_=fmap_p[c * 128:(c + 1) * 128, t * 1024:(t + 1) * 1024])
            for k in range(1, 4):
                nc.sync.dma_start(out=ind_sb[:, k * 1024:(k + 1) * 1024],
                                  in_=ind_p[:, k * 1024:(k + 1) * 1024])
            nc.gpsimd.dma_start(out=relh_sb, in_=relh_p[:, :])
            nc.gpsimd.dma_start(out=relw_sb, in_=relw_p[:, :])
            nc.gpsimd.dma_start(out=onesh_sb, in_=onesh_p[:, :])
            nc.gpsimd.dma_start(out=bias4_sb, in_=bias4_p[:, :])

            QT = big.tile([128, NQ], bf16, name="QT")
            KT = big.tile([128, L], bf16, name="KT")
            VTt = big.tile([128, L], bf16, name="VTt")
            Vn = big.tile([128, L], bf16, name="Vn")
            BT = big.tile([128, NQ], bf16, name="BT")

            # ---- PE warmup: ~4us of dummy matmuls on a memset tile so the
            # HAM clock-gate opens before the first real matmul arrives ----
            warm_sb = work.tile([128, 512], bf16, name="warm_sb", tag="warm")
            nc.gpsimd.memset(warm_sb, 0.0)
            with tc.tile_pool(name="psW", bufs=1, space="PSUM") as psW:
                wps = psW.tile([128, 512], f32, name="warm_ps", tag="warm")
                for _ in range(12):
                    nc.tensor.matmul(wps, warm_sb[:, 0:128], warm_sb,
                                     start=True, stop=True)

            # ---- phase A+B: qkv projection pipelined with fmap stripe DMAs;
            # bias matmuls emitted mid-stream as PE gap-filler ----
            with tc.tile_pool(name="psA", bufs=2, space="PSUM") as psA:
                def qkv_group(dst, col0, t, eng):
                    ps = psA.tile([128, 1024], f32, name="qkv_ps", tag="qkv", bufs=2)
                    for c in range(4):
                        for h in range(2):
                            nc.tensor.matmul(
                                ps[:, h * 512:(h + 1) * 512],
                                W4[c][:, col0:col0 + 128],
                                F4[c][:, t * 1024 + h * 512: t * 1024 + (h + 1) * 512],
                                start=(c == 0), stop=(c == 3))
                    if eng == "act":
                        nc.scalar.copy(dst[:, t * 1024:(t + 1) * 1024], ps)
                    else:
                        nc.vector.tensor_copy(dst[:, t * 1024:(t + 1) * 1024], ps)

                def bias_all():
                    bh_ps = psA.tile([64, NQ], f32, name="bh_ps", tag="bias", bufs=1)
                    for r in range(32):
                        nc.tensor.matmul(
                            bh_ps[:, r * 64:(r + 1) * 64],
                            relh_sb[:, 31 - r:95 - r],
                            QT[:, r * 64:(r + 1) * 64],
                            start=True, stop=True)
                    nc.vector.tensor_copy(BT[0:64, :], bh_ps)
                    bw_ps = psA.tile([64, NQ], f32, name="bw_ps", tag="bias", bufs=1)
                    for w in range(64):
                        nc.tensor.matmul(
                            bw_ps[:, w * 32:(w + 1) * 32],
                            relw_sb[:, 63 - w:127 - w],
                            QT.rearrange("d (i w) -> d w i", w=64)[:, w, :],
                            start=True, stop=True)
                    nc.vector.tensor_copy(
                        BT[64:128, :].rearrange("p (i w) -> p i w", i=32, w=64),
                        bw_ps.rearrange("p (w i) -> p i w", w=64, i=32))

                for t in range(4):
                    if t < 2:
                        qkv_group(QT, 0, t, "dve")
                    qkv_group(KT, 128, t, "act")
                    qkv_group(VTt, 256, t, "dve")
                    for s in range(t * 8, t * 8 + 8):
                        nc.sync.dma_start_transpose(
                            Vn[:, s * 128:(s + 1) * 128],
                            VTt[:, s * 128:(s + 1) * 128])
                    if t == 1:
                        bias_all()

            # ---- phase C: attention main loop ----
            with tc.tile_pool(name="psC", bufs=1, space="PSUM") as psC:
                for qb in range(2):
                    q0 = qb * QB
                    acc = work.tile([128, QB], fp16, name="acc", tag="acc", bufs=2)
                    outT = psC.tile([128, QB], f32, name="outT", tag="out", bufs=1)
                    for kc in range(32):
                        sim = psC.tile([128, QB], f32, name="sim", tag="sim", bufs=3)
                        for h in range(2):
                            sl = slice(q0 + h * 512, q0 + (h + 1) * 512)
                            po = sim[:, h * 512:(h + 1) * 512]
                            nc.tensor.matmul(
                                po, KT[:, kc * 128:(kc + 1) * 128], QT[:, sl],
                                start=True, stop=False)
                            nc.tensor.matmul(
                                po, ind_sb[:, kc * 128:(kc + 1) * 128], BT[:, sl],
                                start=False, stop=True)
                        expT = work.tile([128, QB], fp16, name="expT", tag="exp", bufs=4)
                        nc.scalar.activation(expT, sim, EXPF, bias=bias4_sb[:, 0:1], scale=SCALE)
                        if kc == 0:
                            nc.vector.tensor_copy(acc, expT)
                        else:
                            nc.vector.tensor_add(acc, acc, expT)
                        for h in range(2):
                            nc.tensor.matmul(
                                outT[:, h * 512:(h + 1) * 512],
                                Vn[:, kc * 128:(kc + 1) * 128],
                                expT[:, h * 512:(h + 1) * 512],
                                start=(kc == 0), stop=(kc == 31))

                    # normalize in pipelined 512-wide halves: rowsum
                    # (ones-matmul partition reduce) -> broadcast (K=1 outer
                    # product) -> approx reciprocal -> scale -> store
                    for hh in range(2):
                        sl = slice(hh * 512, (hh + 1) * 512)
                        rs_ps = psC.tile([1, 512], f32, name="rs_ps", tag="sim", bufs=3)
                        nc.tensor.matmul(rs_ps, onesh_sb[:, 0:1], acc[:, sl],
                                         start=True, stop=True)
                        rs_row = work.tile([1, 512], fp16, name="rs_row", tag="rsrow", bufs=2)
                        nc.vector.tensor_copy(rs_row, rs_ps)
                        bc_ps = psC.tile([128, 512], f32, name="bc_ps", tag="sim", bufs=3)
                        nc.tensor.matmul(bc_ps, onesh_sb[0:1, :], rs_row,
                                         start=True, stop=True)
                        rec_sb = work.tile([128, 512], f32, name="rec_sb", tag="bc", bufs=2)
                        nc.vector.reciprocal_approx_fast(out=rec_sb, in_=bc_ps)
                        out_sb = work.tile([128, 512], f32, name="out_sb", tag="osb", bufs=2)
                        nc.vector.tensor_mul(out_sb, outT[:, sl], rec_sb)
                        nc.sync.dma_start(out=out_p[:, q0 + hh * 512:q0 + (hh + 1) * 512],
                                          in_=out_sb)

    nc.finalize()
    return nc


def _prep_core_inputs(fmap, w_qkv, rel_height, rel_width, core):
    bf = ml_dtypes.bfloat16
    h, half = core // 2, core % 2
    q0 = half * NQ
    perm = (np.arange(L) + q0) % L
    fmap_flat = fmap.reshape(C, L)
    fmap_core = np.ascontiguousarray(fmap_flat[:, perm]).astype(bf)
    rows = np.r_[h * 128:(h + 1) * 128,
                 512 + h * 128:512 + (h + 1) * 128,
                 1024 + h * 128:1024 + (h + 1) * 128]
    wt = np.ascontiguousarray(w_qkv[rows].T).astype(bf)
    relhT = rel_height.T  # (128, 127)
    a = 32 * (1 - half)
    relh_slab = np.zeros((128, 96), np.float32)
    relh_slab[:, :95] = relhT[:, a:a + 95]
    relw = np.ascontiguousarray(rel_width.T).astype(bf)
    j = np.arange(L)
    ind = np.zeros((128, L), np.float32)
    ind[(j // 64 + 32 * half) % 64, j] = 1.0
    ind[64 + (j % 64), j] = 1.0
    return {
        "fmapc": fmap_core,
        "wt": wt,
        "relh": relh_slab.astype(bf),
        "relw": relw,
        "ind": ind.astype(bf),
        "onesh": np.ones((128, 128), np.float16),
        "bias4": np.full((128, 1), -4.0, np.float32),
    }


def _install_trace_hook():
    """Register the axon NTFF profiling hook (missing antenv.axon_hooks shim)
    and neuter the artifact upload so tracing works in this sandbox."""
    import sys
    import types
    import concourse.bass_utils as bu
    bu.upload_artifacts = lambda d: d
    try:
        from antenv import axon_hooks  # noqa: F401
        return
    except ImportError:
        pass
    import antenv
    mod = types.ModuleType("antenv.axon_hooks")
    mod._hook = None
    def set_axon_ntff_profile_hook(h):
        mod._hook = h
    def get_axon_ntff_profile_hook():
        return mod._hook
    mod.set_axon_ntff_profile_hook = set_axon_ntff_profile_hook
    mod.get_axon_ntff_profile_hook = get_axon_ntff_profile_hook
    sys.modules["antenv.axon_hooks"] = mod
    antenv.axon_hooks = mod
    try:
        from trn_agent_boot.trn_boot import _ntff_profile_via_ctypes
        h = _ntff_profile_via_ctypes("/opt/axon/libaxon_pjrt.so")
        if h is not None:
            mod._hook = h
    except Exception as e:
        print(f"trace hook install failed: {e}")


def kernel(fmap, w_qkv, rel_height, rel_width, _trace=False):
    global _GRAPH
    from concourse.bass_utils import run_bass_kernel_spmd

    fmap = np.asarray(fmap, dtype=np.float32)
    w_qkv = np.asarray(w_qkv, dtype=np.float32)
    rel_height = np.asarray(rel_height, dtype=np.float32)
    rel_width = np.asarray(rel_width, dtype=np.float32)

    if _GRAPH is None:
        _GRAPH = _build_graph()
    nc = _GRAPH

    in_maps = [_prep_core_inputs(fmap, w_qkv, rel_height, rel_width, c)
               for c in range(NCORES)]
    kw = {}
    if _trace:
        _install_trace_hook()
        import os
        os.makedirs("/tmp/ktrace", exist_ok=True)
        import tempfile
        kw = dict(tmpdir=tempfile.mkdtemp(dir="/tmp/ktrace"))
    res = run_bass_kernel_spmd(nc, in_maps, core_ids=list(range(NCORES)),
                               trace=_trace, **kw)
    out_full = np.zeros((C, L), np.float32)
    for c in range(NCORES):
        h, half = c // 2, c % 2
        out_full[h * 128:(h + 1) * 128, half * NQ:(half + 1) * NQ] = \
            np.asarray(res.results[c]["out"])
    if _trace:
        kernel._last_exec_time_ns = res.exec_time_ns
        kernel._last_profile = res.profile_json
    return out_full.reshape(1, C, H, W)
